# revision 16
# baseline (speedup 1.0000x reference)
"""Neighbor aggregation (GNN message passing) on 8 Trainium2 cores — v22.

vs v21: the idx head (first 32 pairs) lands in its OWN tile so the first
gather's dependency is tile-exact — with a single idx tile the subtile
tracker made the first gather wait for the whole 6.4 MB upload (~18 us of
dead ramp).

vs v19: gather calls use single_packet=False so the SDMA engines can
interleave packets across in-flight calls instead of draining one call's
packet monolithically.

vs v17: deeper gather-tile ring (30 bufs) and the idx upload split into a
small head DMA (first pairs) plus the remainder, so the first gather
starts ~25 us earlier.

vs v16: destination blocks narrowed 128 -> 64 dsts (392 blocks, 8 groups
of 128 edge-slots each).  The on-chip one-hot W area is slots x block
width, so halving the width halves the DVE build work (v16's pacer at
~900 us) while matmul streaming time is unchanged (time ~ moving columns,
not output partitions).  Gather calls keep the proven 1024-descriptor
shape by pairing two blocks' same-window groups per call.

Carried over from v16: per-edge one-hot W built on-chip by per-pair
tensor_tensor (single-port DVE mode — never locks GPSIMD out of the
SWDGE descriptor rings), LPT bin-packing of dsts so every block fits its
fixed slot budget, fixed input-independent program, fp16 output upcast on
host.
"""

import heapq
import sys

sys.path.insert(0, "/opt/trn_rl_repo")

import numpy as np

import concourse.bacc as bacc
import concourse.tile as tile
from concourse import mybir
from concourse.bass_utils import run_bass_kernel_spmd

B = 4
N_NODES = 50000
HS = 16
C = HS * HS
P = 128
BW = 64                       # dsts per block
NBLK = 392
NPAIR = NBLK // 2
NROWS = NBLK * BW             # 25088 packed dst rows per core
GLO = GHI = 4                 # groups per block per window
GPB = GLO + GHI               # 8
GPP = 2 * GPB                 # groups per block-pair, 16
NGRP = NBLK * GPB             # 3136
HI_BASE = 17232               # hi window [17232, 50000); idx = src - 17232
N_QUEUES = 4

_prog_cache: dict = {}
_last_in_maps: list | None = None


def _build_program():
    """Fixed-structure program.  Per block-pair: one lo-window gather call
    (blocks' 4+4 groups, 1024 idxs), one hi-window call, one fused one-hot
    W build [128, 16, 64], then per block 8 accumulating matmuls into a
    [64, 256] PSUM tile; both blocks' results share one out tile/DMA."""
    nc = bacc.Bacc("TRN2", target_bir_lowering=False, debug=False,
                   num_swdge_queues=N_QUEUES)
    h_d = nc.dram_tensor("h", (50048, C), mybir.dt.float16, kind="ExternalInput")
    idx_d = nc.dram_tensor("idx", (P, NGRP * 8), mybir.dt.int16,
                           kind="ExternalInput")
    col_d = nc.dram_tensor("col", (P, NGRP), mybir.dt.float16,
                           kind="ExternalInput")
    wv_d = nc.dram_tensor("wv", (P, NGRP), mybir.dt.float16,
                          kind="ExternalInput")
    iota_d = nc.dram_tensor("iota", (P, GPP * BW), mybir.dt.float16,
                            kind="ExternalInput")
    out_d = nc.dram_tensor("out", (NROWS, C), mybir.dt.float16,
                           kind="ExternalOutput")

    h_ap = h_d.ap()
    win_aps = (h_ap[0:32768, :], h_ap[HI_BASE:HI_BASE + 32768, :])
    q = 0

    with tile.TileContext(nc) as tc:
        with tc.tile_pool(name="const", bufs=1) as cpool, \
             tc.tile_pool(name="gat", bufs=30) as gpool, \
             tc.tile_pool(name="wt", bufs=3) as wpool, \
             tc.tile_pool(name="otile", bufs=4) as opool, \
             tc.tile_pool(name="psum", bufs=8, space="PSUM") as ppool:
            HEADP = 32            # pairs whose idx lands in the head tile
            head = HEADP * GPP * 8
            idxh_t = cpool.tile([P, head], mybir.dt.int16)
            nc.sync.dma_start(out=idxh_t[:], in_=idx_d.ap()[:, :head])
            idx_t = cpool.tile([P, NGRP * 8 - head], mybir.dt.int16)
            nc.sync.dma_start(out=idx_t[:], in_=idx_d.ap()[:, head:])
            col_t = cpool.tile([P, NGRP], mybir.dt.float16)
            nc.sync.dma_start(out=col_t[:], in_=col_d.ap())
            wv_t = cpool.tile([P, NGRP], mybir.dt.float16)
            nc.sync.dma_start(out=wv_t[:], in_=wv_d.ap())
            iota_t = cpool.tile([P, GPP, BW], mybir.dt.float16)
            nc.sync.dma_start(out=iota_t[:], in_=iota_d.ap())

            for pq in range(NPAIR):
                # stream group order within pair: side-major then
                # block-sub-major: g = pq*16 + side*8 + sub*4 + k
                gtiles = []
                for s in (0, 1):
                    pos0 = pq * GPP + s * GPP // 2
                    if pq < HEADP:
                        src_t, c0 = idxh_t, pos0 * 8
                    else:
                        src_t, c0 = idx_t, (pos0 - HEADP * GPP) * 8
                    t = gpool.tile([P, 8, C], mybir.dt.float16, tag="g8")
                    nc.gpsimd.dma_gather(
                        out_ap=t[:],
                        in_ap=win_aps[s],
                        idxs_ap=src_t[:, c0:c0 + 64],
                        num_idxs=8 * P,
                        num_idxs_reg=8 * P,
                        elem_size=C,
                        single_packet=False,
                        queue_num=q % N_QUEUES,
                    )
                    q += 1
                    gtiles.append(t)

                wblk = wpool.tile([P, GPP, BW], mybir.dt.float16, tag="w")
                colb = col_t[:, pq * GPP:(pq + 1) * GPP].broadcast_to(
                    [P, GPP, BW])
                wvb = wv_t[:, pq * GPP:(pq + 1) * GPP].broadcast_to(
                    [P, GPP, BW])
                nc.vector.tensor_tensor(out=wblk[:], in0=iota_t[:], in1=colb,
                                        op=mybir.AluOpType.is_equal)
                nc.vector.tensor_tensor(out=wblk[:], in0=wblk[:], in1=wvb,
                                        op=mybir.AluOpType.mult)

                ot = opool.tile([P, C], mybir.dt.float16, tag="out")
                for sub in (0, 1):
                    acc = ppool.tile([BW, C], mybir.dt.float32, space="PSUM")
                    for k in range(GPB):
                        s, kk = divmod(k, GLO)
                        gslot = s * GPB + sub * GLO + kk
                        nc.tensor.matmul(
                            out=acc[:], lhsT=wblk[:, gslot, :],
                            rhs=gtiles[s][:, sub * GLO + kk, :],
                            start=(k == 0), stop=(k == GPB - 1))
                    nc.scalar.activation(
                        out=ot[sub * BW:(sub + 1) * BW, :], in_=acc[:],
                        func=mybir.ActivationFunctionType.Copy)
                nc.sync.dma_start(out=out_d.ap()[pq * P:(pq + 1) * P, :],
                                  in_=ot[:])

    nc.compile()
    return nc


def _split_halves(deg):
    """Split nodes into two halves balancing edge count; <=NROWS nodes each."""
    order = np.argsort(-deg, kind="stable")
    assign = np.empty(N_NODES, np.int8)
    sums = [0, 0]
    cnts = [0, 0]
    for nid in order:
        h = 0 if (sums[0] <= sums[1] and cnts[0] < NROWS) else 1
        if cnts[h] >= NROWS:
            h = 1 - h
        assign[nid] = h
        sums[h] += int(deg[nid])
        cnts[h] += 1
    return assign


def _pack_blocks(node_ids, deg, cap_edges):
    """LPT: pack nodes into NBLK bins, <=BW nodes and <=cap_edges edges per
    bin.  Returns (blk_of_node, col_of_node) arrays indexed like node_ids."""
    dg = deg[node_ids]
    order = np.argsort(-dg, kind="stable")
    heap = [(0, j) for j in range(NBLK)]
    heapq.heapify(heap)
    binsum = np.zeros(NBLK, np.int64)
    bincnt = np.zeros(NBLK, np.int64)
    blk = np.empty(len(node_ids), np.int64)
    col = np.empty(len(node_ids), np.int64)
    for i in order:
        while True:
            _, j = heapq.heappop(heap)
            if bincnt[j] < BW:
                break
        blk[i] = j
        col[i] = bincnt[j]
        binsum[j] += int(dg[i])
        bincnt[j] += 1
        if bincnt[j] < BW:
            heapq.heappush(heap, (int(binsum[j]), j))
    if binsum.max() > cap_edges:
        # repair pass: swap heavy items out of overflowing bins
        for j in np.where(binsum > cap_edges)[0]:
            items_j = np.where(blk == j)[0]
            for j2 in np.argsort(binsum):
                if binsum[j] <= cap_edges:
                    break
                items_2 = np.where(blk == j2)[0]
                for i1 in items_j[np.argsort(-dg[items_j])]:
                    need = binsum[j] - cap_edges
                    cands = items_2[(dg[items_2] < dg[i1])]
                    if not len(cands):
                        continue
                    i2 = cands[np.argmax(dg[cands])]
                    delta = int(dg[i1] - dg[i2])
                    if delta <= 0 or binsum[j2] + delta > cap_edges:
                        continue
                    blk[i1], blk[i2] = j2, j
                    col[i1], col[i2] = col[i2], col[i1]
                    binsum[j] -= delta
                    binsum[j2] += delta
                    if binsum[j] <= cap_edges:
                        break
        if binsum.max() > cap_edges:
            raise RuntimeError(f"block overflow: {binsum.max()} > {cap_edges}")
    return blk, col


def kernel(H, edge_index, edge_weight, node_idx):
    H = np.asarray(H, dtype=np.float32)
    edge_index = np.asarray(edge_index)
    edge_weight = np.ascontiguousarray(np.asarray(edge_weight), dtype=np.float32)
    node_idx = np.asarray(node_idx)

    inv = np.argsort(node_idx).astype(np.int64)

    LO_CAP = GLO * P              # 512 per block
    iota = np.ascontiguousarray(
        np.tile(np.arange(BW, dtype=np.float16), (P, GPP)))

    in_maps = []
    unpack = []   # (node_ids, rows) per core
    h16_cache = {}
    for core in range(2 * B):
        b, half = divmod(core, 2)
        dst = inv[edge_index[b, :, 0]]
        src = inv[edge_index[b, :, 1]]
        w = edge_weight[b]
        deg = np.bincount(dst, minlength=N_NODES)
        if half == 0:
            _split_halves.cache = _split_halves(deg)
        assign = _split_halves.cache
        node_ids = np.where(assign == half)[0]
        blk_n, col_n = _pack_blocks(node_ids, deg, GPB * P)

        blkof = np.full(N_NODES, -1, np.int64)
        colof = np.full(N_NODES, -1, np.int64)
        blkof[node_ids] = blk_n
        colof[node_ids] = col_n

        m = assign[dst] == half
        d_blk = blkof[dst[m]]
        d_col = colof[dst[m]]
        s = src[m]
        wv = w[m]

        cat = (s >= HI_BASE).astype(np.int64) + (s >= 32768)
        order = np.lexsort((cat, d_blk))
        d_blk = d_blk[order]; d_col = d_col[order]
        s = s[order]; wv = wv[order]; cat = cat[order]

        nblk_e = np.bincount(d_blk, minlength=NBLK)
        nlo = np.bincount(d_blk[cat == 0], minlength=NBLK)
        nlomid = np.bincount(d_blk[cat <= 1], minlength=NBLK)
        if nlo.max() > LO_CAP or (nblk_e - np.minimum(nlomid, LO_CAP)).max() > GHI * P:
            raise RuntimeError("side overflow")
        lo_n = np.clip(LO_CAP, nlo, nlomid)

        starts = np.zeros(NBLK, np.int64)
        starts[1:] = np.cumsum(nblk_e)[:-1]
        rank = np.arange(len(s)) - starts[d_blk]
        side = (rank >= lo_n[d_blk]).astype(np.int64)
        r_side = rank - side * lo_n[d_blk]
        # group stream: pair-major, then side, then block-sub, then k
        g = ((d_blk >> 1) * GPP + side * GPB + (d_blk & 1) * GLO
             + (r_side >> 7))
        p = r_side & 127

        sl = np.zeros(NGRP * P, np.int16)
        sl[g * P + p] = (s - side * HI_BASE).astype(np.int16)
        idx16 = sl.reshape(NGRP, 8, 16).transpose(2, 0, 1).reshape(16, NGRP * 8)
        idx128 = np.ascontiguousarray(np.tile(idx16, (8, 1)))

        colarr = np.zeros((P, NGRP), np.float16)
        warr = np.zeros((P, NGRP), np.float16)
        colarr[p, g] = d_col
        warr[p, g] = wv.astype(np.float16)

        h16 = h16_cache.get(b)
        if h16 is None:
            h16 = np.zeros((50048, C), np.float16)
            h16[:N_NODES] = H[b].reshape(N_NODES, C).astype(np.float16)
            h16_cache[b] = h16

        in_maps.append({"h": h16, "idx": idx128, "col": colarr, "wv": warr,
                        "iota": iota})
        unpack.append((node_ids, blk_n * BW + col_n))

    global _last_in_maps
    _last_in_maps = in_maps
    key = ("v22", BW, GLO, GHI)
    nc = _prog_cache.get(key)
    if nc is None:
        nc = _build_program()
        _prog_cache[key] = nc

    res = run_bass_kernel_spmd(nc, in_maps, list(range(2 * B)))

    out = np.empty((B, N_NODES, HS, HS), np.float32)
    for core in range(2 * B):
        b = core // 2
        node_ids, rows = unpack[core]
        r = res.results[core]["out"].astype(np.float32)
        out[b, node_ids] = r[rows].reshape(-1, HS, HS)
    return out
